# revision 50
# baseline (speedup 1.0000x reference)
"""Trainium2 Bass kernel: batched attention  out = softmax(Q K^T) V  (no 1/sqrt(d) scale).

Shapes (hardcoded): Q, K, V: [4, 16, 2048, 128] fp32 -> out [4, 16, 2048, 128] fp32.

Sharding: B*H = 64 heads, data-parallel across 8 NeuronCores (8 heads per core).

Per-head device algorithm (transpose-free matmul layout):
  Host pre-transposes Q, K to [D, N] per head, fp16 (error budget: fp16
  rounding of Q,K gives ~4.5e-3 output rel err vs the 2e-2 gate). V fp16.
  Queries are processed in 4 quarters of 512 columns; keys in 16 chunks of
  128 rows. Per (quarter, key-chunk):
      S_T[c]  = kc.T @ q            (fp16, one N=512 matmul) -> PSUM fp32
      E       = exp(S_T)            (ACT; bf16; no max-subtract needed)
      O_T    += vc.T @ E[c]         (PSUM accumulate, fp32)
      l4[g]  += ones.T @ E[c],  g = c mod 4   (column-tiled row-sum waves)
  l = mask4.T @ l4;  r = 1/l (DVE); partition-broadcast (GPSIMD); O_T * r.

ACT is the bottleneck engine (exp of all N^2 scores at 1 elem/lane/cycle
@1.2GHz + ~352 cyc fixed cost per ACTIVATE). To amortize the fixed cost,
S PSUM tiles hold 3 key chunks ([128, 3, 512] = 3 banks) and one ACTIVATE
covers 1536 columns. PSUM budget: 2x3 (S, double-buffered) + 1 (O) + 1 (l)
= 8 banks. Key chunks per quarter are grouped [3,3,3,3,2,2].

The whole kernel is issued as a flat software-pipelined stream of units
(one unit = one S-group + its ACTIVATE), with PV matmuls lagging 5 units,
row-sum waves and the normalization tail of round r spread over the first
units of round r+1, so the PE/DVE/GPSIMD work always hides under ACT and
the single-buffered ps_o/ps_l turnarounds never stall the PE.

Measured on trn2 (8 cores): HW exec ~311 us (baseline 499 us), rel err
~1.7e-3 vs fp32 ref (gate 2e-2).
"""

import sys

sys.path.insert(0, "/opt/trn_rl_repo")

import numpy as np

import concourse.bass as bass
import concourse.tile as tile
from concourse import bacc, mybir
from concourse.bass_utils import run_bass_kernel_spmd

B, H, N, D = 4, 16, 2048, 128
NCORES = 8
HPC = (B * H) // NCORES  # heads per core = 8
P = 128                  # partitions
NK = N // P              # key chunks per head = 16
QQ = 4                   # query quarters per head
QW = N // QQ             # 512
NR = HPC * QQ            # rounds per core = 32
GROUPS = [(0, 3), (3, 3), (6, 3), (9, 3), (12, 2), (14, 2)]  # (start chunk, size)
NG = len(GROUPS)
NU = NR * NG             # total units
F32 = mybir.dt.float32
BF16 = mybir.dt.bfloat16
FP16 = mybir.dt.float16
EXP = mybir.ActivationFunctionType.Exp


def build_nc():
    nc = bacc.Bacc(None, target_bir_lowering=False)

    q1_d = nc.dram_tensor("q1", [HPC, D, N], FP16, kind="ExternalInput")
    k1_d = nc.dram_tensor("k1", [HPC, D, N], FP16, kind="ExternalInput")
    v_d = nc.dram_tensor("v", [HPC, N, D], FP16, kind="ExternalInput")
    ot_d = nc.dram_tensor("ot", [HPC, D, N], F32, kind="ExternalOutput")

    with tile.TileContext(nc) as tc:
        with (
            tc.tile_pool(name="const", bufs=1) as const_pool,
            tc.tile_pool(name="io", bufs=2) as io_pool,
            tc.tile_pool(name="e", bufs=12) as e_pool,
            tc.tile_pool(name="osb", bufs=4) as o_pool,
            tc.tile_pool(name="small", bufs=4) as small_pool,
            tc.tile_pool(name="ps_s", bufs=2, space="PSUM") as ps_s_pool,
            tc.tile_pool(name="ps_o", bufs=1, space="PSUM") as ps_o_pool,
            tc.tile_pool(name="ps_l", bufs=1, space="PSUM") as ps_l_pool,
        ):
            ones_col = const_pool.tile([P, 1], FP16)  # sum weights
            nc.vector.memset(ones_col[:], 1.0)
            mask4 = const_pool.tile([P, 1], BF16)     # combine weights
            nc.vector.memset(mask4[:], 0.0)
            for g in range(4):
                nc.vector.memset(mask4[32 * g: 32 * g + 1, :], 1.0)
            # dummy exp to pull the ACT table load into the startup DMA window
            warm = const_pool.tile([P, 1], BF16)
            nc.scalar.activation(warm[:], ones_col[:], EXP)
            # PE clock warm-up: the HAM gate holds the PE at 1.2GHz until it
            # sees ~3.4us of sustained matmul activity, and re-throttles after
            # a ~3.4us idle window. Without this, the startup DMA stall means
            # the first ~6us of real matmuls run at half clock. Dummy N=512
            # matmuls (zeros, M=1 into the ps_l bank -- cleared later by
            # wave0's start=True) keep the PE busy through the DMA window.
            dummy = const_pool.tile([P, 512], FP16, name="dummy")
            nc.vector.memset(dummy[:], 0.0)
            ps_warm = ps_l_pool.tile([P, QW], F32, tag="l", name="ps_warm")
            for _ in range(40):
                nc.tensor.matmul(
                    ps_warm[0:1, :], ones_col[:], dummy[:], start=True, stop=True
                )

            def load_head(h, first=False):
                # split each input DMA so the first S matmuls can start as
                # soon as the first column blocks land; order: q quarter 0
                # and the k blocks first (S stream), then v (PV lags 3
                # units), then the remaining q quarters (next rounds).
                # One DMA engine moves ~22.5 B/ns, so the startup-critical
                # first blocks are striped 128 cols wide across engines.
                q1t = io_pool.tile([P, N], FP16, tag="q1")
                k1t = io_pool.tile([P, N], FP16, tag="k1")
                vt3 = io_pool.tile([P, NK, P], FP16, tag="vt")
                v_r = v_d[h].rearrange("(c p) d -> p c d", p=P)
                if first:
                    # minimal-critical-path order: the first exp needs only
                    # k cols 0-383 (chunks 0-2) and q quarter 0; each DMA
                    # dispatch serializes ~650ns on the sync queue
                    for i in range(3):
                        sl = slice(i * P, (i + 1) * P)
                        nc.sync.dma_start(out=k1t[:, sl], in_=k1_d[h][:, sl])
                    nc.sync.dma_start(out=q1t[:, 0:256], in_=q1_d[h][:, 0:256])
                    nc.sync.dma_start(out=q1t[:, 256:512], in_=q1_d[h][:, 256:512])
                    nc.sync.dma_start(out=k1t[:, 384:512], in_=k1_d[h][:, 384:512])
                else:
                    nc.sync.dma_start(out=q1t[:, 0:512], in_=q1_d[h][:, 0:512])
                    nc.sync.dma_start(out=k1t[:, 0:512], in_=k1_d[h][:, 0:512])
                for i in range(1, 4):
                    sl = slice(i * 512, (i + 1) * 512)
                    nc.sync.dma_start(out=k1t[:, sl], in_=k1_d[h][:, sl])
                for i in range(4):
                    nc.sync.dma_start(
                        out=vt3[:, i * 4: (i + 1) * 4, :],
                        in_=v_r[:, i * 4: (i + 1) * 4, :],
                    )
                for i in range(1, 4):
                    sl = slice(i * 512, (i + 1) * 512)
                    nc.sync.dma_start(out=q1t[:, sl], in_=q1_d[h][:, sl])
                return q1t, k1t, vt3.rearrange("p c d -> p (c d)")

            head_tiles = {}
            state = {}

            def do_s_act(u):
                r, g = divmod(u, NG)
                h, qq = divmod(r, QQ)
                if g == 0:
                    if qq == 0:
                        head_tiles[h] = load_head(h, first=(h == 0))
                        head_tiles.pop(h - 1, None)
                    state[r] = {
                        "ps_o": ps_o_pool.tile([P, QW], F32, tag="o", name="ps_o"),
                        "ps_l": ps_l_pool.tile([P, QW], F32, tag="l", name="ps_l"),
                        "e": {},
                        "tiles": head_tiles[h],
                        "q0": qq * QW,
                        "h": h,
                    }
                st = state[r]
                q1t, k1t, _ = st["tiles"]
                c0, m = GROUPS[g]
                ps_s = ps_s_pool.tile([P, 3, QW], F32, tag="s")
                for i in range(m):
                    c = c0 + i
                    nc.tensor.matmul(
                        ps_s[:, i, :],
                        k1t[:, c * P: (c + 1) * P],
                        q1t[:, st["q0"]: st["q0"] + QW],
                        start=True,
                        stop=True,
                    )
                e = e_pool.tile([P, 3, QW], BF16, tag="e")
                nc.scalar.activation(e[:, 0:m, :], ps_s[:, 0:m, :], EXP)
                for i in range(m):
                    st["e"][c0 + i] = e[:, i, :]

            def do_pv(u):
                r, g = divmod(u, NG)
                st = state[r]
                vt = st["tiles"][2]
                c0, m = GROUPS[g]
                for i in range(m):
                    c = c0 + i
                    nc.tensor.matmul(
                        st["ps_o"][:],
                        vt[:, c * P: (c + 1) * P],
                        st["e"][c],
                        start=(c == 0),
                        stop=(c == NK - 1),
                    )

            def do_wave(r, w):
                # 4 concurrent M=1 matmuls on distinct PE column groups
                st = state[r]
                for g4 in range(4):
                    c = w * 4 + g4
                    nc.tensor.matmul(
                        st["ps_l"][32 * g4: 32 * g4 + 1, :],
                        ones_col[:],
                        st["e"][c],
                        start=(w == 0),
                        stop=(w == 3),
                        tile_position=(0, 32 * g4),
                    )

            def tail_a(r):  # snapshot l4 off PSUM (DVE)
                st = state[r]
                l4 = small_pool.tile([P, QW], BF16, tag="l4")
                nc.vector.tensor_copy(l4[:], st["ps_l"][:])
                st["l4"] = l4

            def tail_b(r):  # combine quad-rows (PE) -> 1/l (DVE) -> bcast (GPSIMD)
                # the combine writes back into ps_l (whose contents are dead
                # once l4 is snapshotted) rather than taking a ps_s pool slot
                # -- a ps_s allocation here would shift the S double-buffer
                # rotation and serialize S groups against their own ACTIVATE
                st = state[r]
                nc.tensor.matmul(
                    st["ps_l"][0:1, :], mask4[:], st["l4"][:], start=True, stop=True
                )
                r_sb = small_pool.tile([1, QW], F32, tag="r")
                nc.vector.reciprocal_approx_fast(r_sb[:], st["ps_l"][0:1, :])
                r_bc = small_pool.tile([P, QW], F32, tag="rbc")
                nc.gpsimd.partition_broadcast(r_bc[:], r_sb[:])
                st["r_bc"] = r_bc

            def tail_c(r):  # normalize (DVE) + store
                st = state.pop(r)
                o_sb = o_pool.tile([P, QW], F32, tag="osb")
                nc.vector.tensor_mul(o_sb[:], st["ps_o"][:], st["r_bc"][:])
                q0 = st["q0"]
                # split spreads the 256KB store over multiple DMA engines
                # (one engine moves ~22.5 B/ns) so the final store doesn't
                # serialize into the kernel drain
                nway = 2
                w = QW // nway
                for i in range(nway):
                    nc.sync.dma_start(
                        out=ot_d[st["h"]][:, q0 + i * w: q0 + (i + 1) * w],
                        in_=o_sb[:, i * w: (i + 1) * w],
                    )

            for u in range(NU + 6):
                if u < NU:
                    do_s_act(u)
                r, g = divmod(u, NG)
                # row-sum waves: wave w of round r needs ACT of chunks
                # 4w..4w+3, complete after group [g1,g2,g3,g5][w] of r
                if g == 0 and r - 1 >= 0 and u <= NU:
                    do_wave(r - 1, 3)
                    tail_a(r - 1)
                elif g in (3, 4, 5) and r < NR:
                    do_wave(r, g - 3)
                if u - 6 >= 0 and u - 6 < NU:
                    do_pv(u - 6)
                # tail_b right after g0's PV: the combine matmul reaches the
                # PE after ~1.8us (l4 copy done by then, no PE head-block)
                # and the recip+broadcast chain finishes before tail_c's mul
                # is reached at g2 -- so the next round's first PV matmul
                # (g3, needs ps_o freed by that mul) never stalls
                if g == 0 and r - 1 >= 0:
                    tail_b(r - 1)
                if g == 5 and r - 1 >= 0 and u <= NU + 5:
                    tail_c(r - 1)
            # drain: final round's wave3/tail ran in the u-loop overhang
            # (units NU..NU+2 map to g==0,1,2 of a virtual round NR)
    nc.finalize()
    return nc


def _prepare_in_maps(Q, K, V):
    Qf = np.asarray(Q, dtype=np.float32).reshape(B * H, N, D)
    Kf = np.asarray(K, dtype=np.float32).reshape(B * H, N, D)
    Vf = np.asarray(V, dtype=np.float32).reshape(B * H, N, D).astype(np.float16)
    q1 = np.ascontiguousarray(Qf.transpose(0, 2, 1)).astype(np.float16)
    k1 = np.ascontiguousarray(Kf.transpose(0, 2, 1)).astype(np.float16)
    in_maps = []
    for i in range(NCORES):
        s = slice(i * HPC, (i + 1) * HPC)
        in_maps.append({"q1": q1[s], "k1": k1[s], "v": Vf[s]})
    return in_maps


def run(Q, K, V, trace=False, **kwargs):
    nc = build_nc()
    in_maps = _prepare_in_maps(Q, K, V)
    res = run_bass_kernel_spmd(nc, in_maps, list(range(NCORES)), trace=trace, **kwargs)
    OT = np.concatenate([res.results[i]["ot"] for i in range(NCORES)], axis=0)
    out = OT.transpose(0, 2, 1).reshape(B, H, N, D)
    return np.ascontiguousarray(out), res


def kernel(Q, K, V):
    out, _ = run(Q, K, V, trace=False)
    return out


# revision 51
# speedup vs baseline: 1.0210x; 1.0210x over previous
"""Trainium2 Bass kernel: batched attention  out = softmax(Q K^T) V  (no 1/sqrt(d) scale).

Shapes (hardcoded): Q, K, V: [4, 16, 2048, 128] fp32 -> out [4, 16, 2048, 128] fp32.

Sharding: B*H = 64 heads, data-parallel across 8 NeuronCores (8 heads per core).

Per-head device algorithm (transpose-free matmul layout):
  Host pre-transposes Q, K to [D, N] per head, fp16 (error budget: fp16
  rounding of Q,K gives ~4.5e-3 output rel err vs the 2e-2 gate). V fp16.
  Queries are processed in 4 quarters of 512 columns; keys in 16 chunks of
  128 rows. Per (quarter, key-chunk):
      S_T[c]  = kc.T @ q            (fp16, one N=512 matmul) -> PSUM fp32
      E       = exp(S_T)            (ACT; bf16; no max-subtract needed)
      O_T    += vc.T @ E[c]         (PSUM accumulate, fp32)
      l4[g]  += ones.T @ E[c],  g = c mod 4   (column-tiled row-sum waves)
  l = mask4.T @ l4;  r = 1/l (DVE); partition-broadcast (GPSIMD); O_T * r.

ACT is the bottleneck engine (exp of all N^2 scores at 1 elem/lane/cycle
@1.2GHz + ~352 cyc fixed cost per ACTIVATE). To amortize the fixed cost,
S PSUM tiles hold 3 key chunks ([128, 3, 512] = 3 banks) and one ACTIVATE
covers 1536 columns. PSUM budget: 2x3 (S, double-buffered) + 1 (O) + 1 (l)
= 8 banks. Key chunks per quarter are grouped [3,3,3,3,2,2].

The whole kernel is issued as a flat software-pipelined stream of units
(one unit = one S-group + its ACTIVATE), with PV matmuls lagging 5 units,
row-sum waves and the normalization tail of round r spread over the first
units of round r+1, so the PE/DVE/GPSIMD work always hides under ACT and
the single-buffered ps_o/ps_l turnarounds never stall the PE.

Measured on trn2 (8 cores): HW exec ~311 us (baseline 499 us), rel err
~1.7e-3 vs fp32 ref (gate 2e-2).
"""

import sys

sys.path.insert(0, "/opt/trn_rl_repo")

import numpy as np

import concourse.bass as bass
import concourse.tile as tile
from concourse import bacc, mybir
from concourse.bass_utils import run_bass_kernel_spmd

B, H, N, D = 4, 16, 2048, 128
NCORES = 8
HPC = (B * H) // NCORES  # heads per core = 8
P = 128                  # partitions
NK = N // P              # key chunks per head = 16
QQ = 4                   # query quarters per head
QW = N // QQ             # 512
NR = HPC * QQ            # rounds per core = 32
GROUPS = [(0, 3), (3, 3), (6, 3), (9, 3), (12, 2), (14, 2)]  # (start chunk, size)
NG = len(GROUPS)
NU = NR * NG             # total units
F32 = mybir.dt.float32
BF16 = mybir.dt.bfloat16
FP16 = mybir.dt.float16
EXP = mybir.ActivationFunctionType.Exp


def build_nc():
    nc = bacc.Bacc(None, target_bir_lowering=False)

    q1_d = nc.dram_tensor("q1", [HPC, D, N], FP16, kind="ExternalInput")
    k1_d = nc.dram_tensor("k1", [HPC, D, N], FP16, kind="ExternalInput")
    v_d = nc.dram_tensor("v", [HPC, N, D], FP16, kind="ExternalInput")
    ot_d = nc.dram_tensor("ot", [HPC, D, N], F32, kind="ExternalOutput")

    with tile.TileContext(nc) as tc:
        with (
            tc.tile_pool(name="const", bufs=1) as const_pool,
            tc.tile_pool(name="io", bufs=2) as io_pool,
            tc.tile_pool(name="e", bufs=12) as e_pool,
            tc.tile_pool(name="osb", bufs=4) as o_pool,
            tc.tile_pool(name="small", bufs=4) as small_pool,
            tc.tile_pool(name="ps_s", bufs=2, space="PSUM") as ps_s_pool,
            tc.tile_pool(name="ps_o", bufs=1, space="PSUM") as ps_o_pool,
            tc.tile_pool(name="ps_l", bufs=1, space="PSUM") as ps_l_pool,
        ):
            ones_col = const_pool.tile([P, 1], FP16)  # sum weights
            nc.vector.memset(ones_col[:], 1.0)
            mask4 = const_pool.tile([P, 1], BF16)     # combine weights
            nc.vector.memset(mask4[:], 0.0)
            for g in range(4):
                nc.vector.memset(mask4[32 * g: 32 * g + 1, :], 1.0)
            # dummy exp to pull the ACT table load into the startup DMA window
            warm = const_pool.tile([P, 1], BF16)
            nc.scalar.activation(warm[:], ones_col[:], EXP)

            def load_head(h, first=False):
                # split each input DMA so the first S matmuls can start as
                # soon as the first column blocks land; order: q quarter 0
                # and the k blocks first (S stream), then v (PV lags 3
                # units), then the remaining q quarters (next rounds).
                # One DMA engine moves ~22.5 B/ns, so the startup-critical
                # first blocks are striped 128 cols wide across engines.
                q1t = io_pool.tile([P, N], FP16, tag="q1")
                k1t = io_pool.tile([P, N], FP16, tag="k1")
                vt3 = io_pool.tile([P, NK, P], FP16, tag="vt")
                v_r = v_d[h].rearrange("(c p) d -> p c d", p=P)
                if first:
                    # minimal-critical-path order: the first exp needs only
                    # k cols 0-383 (chunks 0-2) and q quarter 0; each DMA
                    # dispatch serializes ~650ns on the sync queue
                    for i in range(3):
                        sl = slice(i * P, (i + 1) * P)
                        nc.sync.dma_start(out=k1t[:, sl], in_=k1_d[h][:, sl])
                    nc.sync.dma_start(out=q1t[:, 0:256], in_=q1_d[h][:, 0:256])
                    nc.sync.dma_start(out=q1t[:, 256:512], in_=q1_d[h][:, 256:512])
                    nc.sync.dma_start(out=k1t[:, 384:512], in_=k1_d[h][:, 384:512])
                else:
                    nc.sync.dma_start(out=q1t[:, 0:512], in_=q1_d[h][:, 0:512])
                    nc.sync.dma_start(out=k1t[:, 0:512], in_=k1_d[h][:, 0:512])
                for i in range(1, 4):
                    sl = slice(i * 512, (i + 1) * 512)
                    nc.sync.dma_start(out=k1t[:, sl], in_=k1_d[h][:, sl])
                for i in range(4):
                    nc.sync.dma_start(
                        out=vt3[:, i * 4: (i + 1) * 4, :],
                        in_=v_r[:, i * 4: (i + 1) * 4, :],
                    )
                for i in range(1, 4):
                    sl = slice(i * 512, (i + 1) * 512)
                    nc.sync.dma_start(out=q1t[:, sl], in_=q1_d[h][:, sl])
                return q1t, k1t, vt3.rearrange("p c d -> p (c d)")

            head_tiles = {}
            state = {}

            def do_s_act(u):
                r, g = divmod(u, NG)
                h, qq = divmod(r, QQ)
                if g == 0:
                    if qq == 0:
                        head_tiles[h] = load_head(h, first=(h == 0))
                        head_tiles.pop(h - 1, None)
                    state[r] = {
                        "ps_o": ps_o_pool.tile([P, QW], F32, tag="o", name="ps_o"),
                        "ps_l": ps_l_pool.tile([P, QW], F32, tag="l", name="ps_l"),
                        "e": {},
                        "tiles": head_tiles[h],
                        "q0": qq * QW,
                        "h": h,
                    }
                st = state[r]
                q1t, k1t, _ = st["tiles"]
                c0, m = GROUPS[g]
                ps_s = ps_s_pool.tile([P, 3, QW], F32, tag="s")
                for i in range(m):
                    c = c0 + i
                    nc.tensor.matmul(
                        ps_s[:, i, :],
                        k1t[:, c * P: (c + 1) * P],
                        q1t[:, st["q0"]: st["q0"] + QW],
                        start=True,
                        stop=True,
                    )
                e = e_pool.tile([P, 3, QW], BF16, tag="e")
                nc.scalar.activation(e[:, 0:m, :], ps_s[:, 0:m, :], EXP)
                for i in range(m):
                    st["e"][c0 + i] = e[:, i, :]

            def do_pv(u):
                r, g = divmod(u, NG)
                st = state[r]
                vt = st["tiles"][2]
                c0, m = GROUPS[g]
                for i in range(m):
                    c = c0 + i
                    nc.tensor.matmul(
                        st["ps_o"][:],
                        vt[:, c * P: (c + 1) * P],
                        st["e"][c],
                        start=(c == 0),
                        stop=(c == NK - 1),
                    )

            def do_wave(r, w):
                # 4 concurrent M=1 matmuls on distinct PE column groups
                st = state[r]
                for g4 in range(4):
                    c = w * 4 + g4
                    nc.tensor.matmul(
                        st["ps_l"][32 * g4: 32 * g4 + 1, :],
                        ones_col[:],
                        st["e"][c],
                        start=(w == 0),
                        stop=(w == 3),
                        tile_position=(0, 32 * g4),
                    )

            def tail_a(r):  # snapshot l4 off PSUM (DVE)
                st = state[r]
                l4 = small_pool.tile([P, QW], BF16, tag="l4")
                nc.vector.tensor_copy(l4[:], st["ps_l"][:])
                st["l4"] = l4

            def tail_b(r):  # combine quad-rows (PE) -> 1/l (DVE) -> bcast (GPSIMD)
                # the combine writes back into ps_l (whose contents are dead
                # once l4 is snapshotted) rather than taking a ps_s pool slot
                # -- a ps_s allocation here would shift the S double-buffer
                # rotation and serialize S groups against their own ACTIVATE
                st = state[r]
                nc.tensor.matmul(
                    st["ps_l"][0:1, :], mask4[:], st["l4"][:], start=True, stop=True
                )
                r_sb = small_pool.tile([1, QW], F32, tag="r")
                nc.vector.reciprocal_approx_fast(r_sb[:], st["ps_l"][0:1, :])
                r_bc = small_pool.tile([P, QW], F32, tag="rbc")
                nc.gpsimd.partition_broadcast(r_bc[:], r_sb[:])
                st["r_bc"] = r_bc

            def tail_c(r):  # normalize (DVE) + store
                st = state.pop(r)
                o_sb = o_pool.tile([P, QW], F32, tag="osb")
                nc.vector.tensor_mul(o_sb[:], st["ps_o"][:], st["r_bc"][:])
                q0 = st["q0"]
                # split spreads the 256KB store over multiple DMA engines
                # (one engine moves ~22.5 B/ns) so the final store doesn't
                # serialize into the kernel drain
                nway = 2
                w = QW // nway
                for i in range(nway):
                    nc.sync.dma_start(
                        out=ot_d[st["h"]][:, q0 + i * w: q0 + (i + 1) * w],
                        in_=o_sb[:, i * w: (i + 1) * w],
                    )

            for u in range(NU + 6):
                if u < NU:
                    do_s_act(u)
                r, g = divmod(u, NG)
                # row-sum waves: wave w of round r needs ACT of chunks
                # 4w..4w+3, complete after group [g1,g2,g3,g5][w] of r
                if g == 0 and r - 1 >= 0 and u <= NU:
                    do_wave(r - 1, 3)
                    tail_a(r - 1)
                elif g in (3, 4, 5) and r < NR:
                    do_wave(r, g - 3)
                if u - 6 >= 0 and u - 6 < NU:
                    do_pv(u - 6)
                # tail_b right after g0's PV: the combine matmul reaches the
                # PE after ~1.8us (l4 copy done by then, no PE head-block)
                # and the recip+broadcast chain finishes before tail_c's mul
                # is reached at g2 -- so the next round's first PV matmul
                # (g3, needs ps_o freed by that mul) never stalls
                if g == 0 and r - 1 >= 0:
                    tail_b(r - 1)
                if g == 5 and r - 1 >= 0 and u <= NU + 5:
                    tail_c(r - 1)
            # drain: final round's wave3/tail ran in the u-loop overhang
            # (units NU..NU+2 map to g==0,1,2 of a virtual round NR)
    nc.finalize()
    return nc


def _prepare_in_maps(Q, K, V):
    Qf = np.asarray(Q, dtype=np.float32).reshape(B * H, N, D)
    Kf = np.asarray(K, dtype=np.float32).reshape(B * H, N, D)
    Vf = np.asarray(V, dtype=np.float32).reshape(B * H, N, D).astype(np.float16)
    q1 = np.ascontiguousarray(Qf.transpose(0, 2, 1)).astype(np.float16)
    k1 = np.ascontiguousarray(Kf.transpose(0, 2, 1)).astype(np.float16)
    in_maps = []
    for i in range(NCORES):
        s = slice(i * HPC, (i + 1) * HPC)
        in_maps.append({"q1": q1[s], "k1": k1[s], "v": Vf[s]})
    return in_maps


def run(Q, K, V, trace=False, **kwargs):
    nc = build_nc()
    in_maps = _prepare_in_maps(Q, K, V)
    res = run_bass_kernel_spmd(nc, in_maps, list(range(NCORES)), trace=trace, **kwargs)
    OT = np.concatenate([res.results[i]["ot"] for i in range(NCORES)], axis=0)
    out = OT.transpose(0, 2, 1).reshape(B, H, N, D)
    return np.ascontiguousarray(out), res


def kernel(Q, K, V):
    out, _ = run(Q, K, V, trace=False)
    return out
